# revision 49
# baseline (speedup 1.0000x reference)
"""AdaptiveGeometryAttention distributed Bass kernel for 8 trn2 NeuronCores.

Sharding: data-parallel over B (2 groups of 4 cores), head-parallel over H
(4 heads per core). Each core computes its heads' attention and a partial
out-projection [T, C]; a ReduceScatter(add) over each 4-core group leaves
each core with a distinct 256-row shard of the final output, which the host
reassembles.

Self-contained: hardcodes all shapes; host side only shards/transposes
inputs and concatenates the output shards.
"""
import os
import sys

for _p in ("/opt/trn_rl_repo",):
    if _p not in sys.path:
        sys.path.append(_p)

import numpy as np
import concourse.bass as bass
import concourse.bacc as bacc
import concourse.mybir as mybir
from concourse import masks
from concourse.alu_op_type import AluOpType
from concourse.tile import TileContext
from concourse.bass_utils import run_bass_kernel_spmd

AF = mybir.ActivationFunctionType
DT = mybir.dt

B, T, C, H, D = 2, 1024, 1024, 16, 64
HL = 4                 # heads per core
JD = HL * D            # 256 local head dims
N_CORES = 8
GROUPS = [[0, 1, 2, 3], [4, 5, 6, 7]]
SQD = 0.125            # 1/sqrt(D)
NEG = -1.0e30

# dtype knobs
PROJ_F32R = True       # q/k/v/ia projection matmuls via float32r operands
NI_F32R = True         # neg_inner matmul via float32r operands

KSTATS = {}

# The act-table-load placement pass picks the FIRST set containing each
# activation function, so alternating Ln/Exp thrashes between the
# single-function sets (~50 table reloads per kernel). Strip ln/exp from
# those sets so the combined natural_log_exp_and_others set is chosen.
_orig_get_tables = bacc.get_activation_tables


def _patched_get_tables(arch):
    t = _orig_get_tables(arch)
    for nm in ("exp_and_others", "natural_log", "exp_and_friends"):
        if nm in t:
            t[nm] = t[nm] - {AF.Exp, AF.Ln}
    return t


bacc.get_activation_tables = _patched_get_tables


def _f32r(ap):
    return ap.bitcast(DT.float32r)


def _mmdt(ap, use_f32r):
    return _f32r(ap) if use_f32r else ap


def build_nc():
    nc = bacc.Bacc("TRN2")

    # ---- I/O ----
    DT_PROJ = DT.float32r if PROJ_F32R else DT.float32
    DT_NI = DT.float32r if NI_F32R else DT.float32
    JQ = JD + 6            # q proj cols + [imp, alpha*4, pad] cols
    xT_e = nc.dram_tensor("xT", [C, T], DT_PROJ, kind="ExternalInput")
    wqiT_e = nc.dram_tensor("wqiT", [C, JQ], DT_PROJ, kind="ExternalInput")
    wkT_e = nc.dram_tensor("wkT", [C, JD], DT_PROJ, kind="ExternalInput")
    wvT_e = nc.dram_tensor("wvT", [C, JD], DT_PROJ, kind="ExternalInput")
    bq_e = nc.dram_tensor("bq_b", [128, JD], DT.float32, kind="ExternalInput")
    bk_e = nc.dram_tensor("bk_b", [128, JD], DT.float32, kind="ExternalInput")
    bv_e = nc.dram_tensor("bv_b", [128, JD], DT.float32, kind="ExternalInput")
    bia_e = nc.dram_tensor("bia_b", [128, 5], DT.float32, kind="ExternalInput")
    woT_e = nc.dram_tensor("woT", [JD, C], DT.bfloat16, kind="ExternalInput")
    bout_e = nc.dram_tensor("bout_b", [128, C], DT.float32, kind="ExternalInput")
    thneg_e = nc.dram_tensor("thneg_b", [128, 1], DT.float32, kind="ExternalInput")
    cmask_e = nc.dram_tensor("cmask", [128, 128], DT.float32, kind="ExternalInput")
    out_e = nc.dram_tensor("out", [T // 4, C], DT.float32, kind="ExternalOutput")

    # per-chunk staging: separate dram tensors so Tile's per-tensor dependency
    # tracking lets chunk k+1's partial write overlap chunk k's collective
    # (a single tensor serializes: write-after-read on the whole tensor).
    # Chunks pair tis by completion order (7,0),(6,1),(5,2),(4,3).
    RS_PAIRS = [(7, 0), (6, 1), (5, 2), (4, 3)]
    partial_ds = [nc.dram_tensor(f"partial{p}", [256, C], DT.bfloat16)
                  for p in range(4)]
    rs_out_ds = [nc.dram_tensor(f"rsout{p}", [64, C], DT.bfloat16)
                 for p in range(4)]
    # tiny warm-up collective: absorbs inter-core launch skew so the real
    # reduce-scatters don't each pay a peer-wait mid-pipeline
    warm_in_d = nc.dram_tensor("warm_in", [4, 128], DT.float32)
    warm_out_d = nc.dram_tensor("warm_out", [1, 128], DT.float32)

    with TileContext(nc) as tc:
        # The scheduler serializes every DMA-transpose issued after a
        # collective against that collective's completion, which stalls the
        # attention pipeline (~15us per reduce-scatter chunk). Our transposes
        # (SBUF->SBUF, core DMA queues) are independent of the HBM->HBM
        # reduce-scatter chunks, so drop that edge.
        class _NoAddSet(set):
            def add(self, _):
                pass

        tc.serialize_transpose_collective_names = _NoAddSet()
        with (
            tc.tile_pool(name="const", bufs=1) as cpool,
            tc.tile_pool(name="mainp", bufs=1) as mp,
        ):
            # ---- warm-up collective (see warm_in_d) ----
            nc.gpsimd.collective_compute(
                "ReduceScatter", mybir.AluOpType.add,
                replica_groups=GROUPS,
                ins=[warm_in_d[:]],
                outs=[warm_out_d[:]],
            )
            # ---- constants ----
            idf = cpool.tile([128, 128], DT.float32, tag="idf")
            masks.make_identity(nc, idf[:])
            idbf = cpool.tile([128, 128], DT.bfloat16, tag="idbf")
            masks.make_identity(nc, idbf[:])
            cmask = cpool.tile([128, 128], DT.float32, tag="cmask")
            nc.sync.dma_start(out=cmask[:], in_=cmask_e[:])
            czm = cpool.tile([128, 1152], DT.float32, tag="czm")
            nc.vector.memset(czm[:, 0:1024], 0.0)
            nc.vector.tensor_copy(czm[:, 1024:1152], cmask[:])
            bout_b = cpool.tile([128, C], DT.float32, tag="boutb")
            thneg = cpool.tile([128, 1], DT.float32, tag="thneg")
            nc.sync.dma_start(out=thneg[:], in_=thneg_e[:])

            # ---- persistent main tiles ----
            vbf = mp.tile([128, 8 * JD], DT.bfloat16, tag="vbf")
            qbT = [mp.tile([128, T], DT.bfloat16, tag=f"qbT{j}", name=f"qbT{j}") for j in range(2)]
            kbT = [mp.tile([128, T], DT.bfloat16, tag=f"kbT{j}", name=f"kbT{j}") for j in range(2)]
            qhT = [mp.tile([128, T], DT_NI, tag=f"qhT{j}", name=f"qhT{j}") for j in range(2)]
            khT = [mp.tile([128, T], DT_NI, tag=f"khT{j}", name=f"khT{j}") for j in range(2)]
            wobf = mp.tile([128, 2 * C], DT.bfloat16, tag="wobf")
            nalpha = mp.tile([128, 32], DT.float32, tag="nalpha")
            oma = mp.tile([128, 32], DT.float32, tag="oma")
            spike = mp.tile([128, 8], DT.float32, tag="spike")
            # row stats, col = side*32 + ti*4 + h
            rat = mp.tile([128, 64], DT.float32, tag="rat")    # sinh(n)/n
            ratx2 = mp.tile([128, 32], DT.float32, tag="ratx2")  # 2*rat (q side)
            gq = mp.tile([128, 32], DT.float32, tag="gq")      # cosh/ratio (q side)
            coshk = mp.tile([128, 32], DT.float32, tag="coshk")
            nrk = mp.tile([128, 32], DT.float32, tag="nrk")    # -ratio_k

            with tc.tile_pool(name="wpool", bufs=1) as wp, \
                 tc.tile_pool(name="ps1", bufs=8, space="PSUM") as ps1:
                # ---- loads: interleave x and weight chunks so the kc-outer
                # projection matmuls can start as soon as chunk 0 lands ----
                xT = wp.tile([128, 8 * T], DT_PROJ, tag="xT")
                wqi = wp.tile([128, 8 * JQ], DT_PROJ, tag="wqi")
                wk = wp.tile([128, 8 * JD], DT_PROJ, tag="wk")
                wv = wp.tile([128, 8 * JD], DT_PROJ, tag="wv")
                for kc in range(8):
                    # 4 sub-transfers per x chunk: spreads DMA queues and lets
                    # the per-t8 projection matmuls start per quarter-chunk
                    for tq in range(4):
                        nc.sync.dma_start(
                            out=xT[:, kc * T + tq * 256: kc * T + (tq + 1) * 256],
                            in_=xT_e[kc * 128:(kc + 1) * 128,
                                     tq * 256:(tq + 1) * 256],
                        )
                    nc.sync.dma_start(
                        out=wqi[:, kc * JQ:(kc + 1) * JQ],
                        in_=wqiT_e[kc * 128:(kc + 1) * 128, :],
                    )
                    nc.sync.dma_start(
                        out=wk[:, kc * JD:(kc + 1) * JD],
                        in_=wkT_e[kc * 128:(kc + 1) * 128, :],
                    )
                    nc.sync.dma_start(
                        out=wv[:, kc * JD:(kc + 1) * JD],
                        in_=wvT_e[kc * 128:(kc + 1) * 128, :],
                    )
                bq_b = wp.tile([128, JD], DT.float32, tag="bqb")
                bk_b = wp.tile([128, JD], DT.float32, tag="bkb")
                bv_b = wp.tile([128, JD], DT.float32, tag="bvb")
                bia_b = wp.tile([128, 5], DT.float32, tag="biab")
                nc.sync.dma_start(out=bq_b[:], in_=bq_e[:])
                nc.sync.dma_start(out=bk_b[:], in_=bk_e[:])
                nc.sync.dma_start(out=bv_b[:], in_=bv_e[:])
                nc.sync.dma_start(out=bia_b[:], in_=bia_e[:])
                for cc in range(2):
                    nc.sync.dma_start(
                        out=wobf[:, cc * C:(cc + 1) * C],
                        in_=woT_e[cc * 128:(cc + 1) * 128, :],
                    )
                nc.sync.dma_start(out=bout_b[:], in_=bout_e[:])

                # ---- phase 1: projections, kc-outer over 8 live psum tiles
                # (row layout [t, d']); ia cols ride along with q ----
                qrow = wp.tile([128, 8 * JD], DT.float32, tag="qrow")
                krow = wp.tile([128, 8 * JD], DT.float32, tag="krow")
                ia_sb = wp.tile([128, 8 * 5], DT.float32, tag="iasb")

                for pnm, (dst, w_t, b_t, jw) in enumerate((
                    (qrow, wqi, bq_b, JQ),
                    (krow, wk, bk_b, JD),
                    (vbf, wv, bv_b, JD),
                )):
                    pj = [ps1.tile([128, JQ], DT.float32, tag="pj", bufs=8,
                                   name=f"pj{pnm}_{t8}")
                          for t8 in range(8)]
                    for kc in range(8):
                        for t8 in range(8):
                            nc.tensor.matmul(
                                pj[t8][:, 0:jw],
                                xT[:, kc * T + t8 * 128: kc * T + t8 * 128 + 128],
                                w_t[:, kc * jw:(kc + 1) * jw],
                                start=(kc == 0), stop=(kc == 7),
                            )
                    for t8 in range(8):
                        nc.vector.tensor_add(
                            dst[:, t8 * JD:(t8 + 1) * JD], pj[t8][:, 0:JD], b_t[:]
                        )
                        if jw == JQ:
                            tmp5 = wp.tile([128, 5], DT.float32, tag="tmp5")
                            nc.vector.tensor_add(tmp5[:], pj[t8][:, JD:JD + 5],
                                                 bia_b[:])
                            nc.scalar.activation(ia_sb[:, t8 * 5:(t8 + 1) * 5],
                                                 tmp5[:], AF.Sigmoid)

                # ---- spike / nalpha ----
                ia3 = ia_sb[:].rearrange("p (t f) -> p t f", f=5)
                # importance[t=0] := 0 (cmask) before threshold compare
                nc.vector.memset(ia_sb[0:1, 0:1], 0.0)
                imp8 = wp.tile([128, 8], DT.float32, tag="imp8")
                nc.vector.tensor_scalar(imp8[:], ia3[:, :, 0:1], thneg[:], None, AluOpType.add)
                sgn8 = wp.tile([128, 8], DT.float32, tag="sgn8")
                nc.scalar.activation(sgn8[:], imp8[:], AF.Sign)
                nc.vector.tensor_scalar_max(spike[:], sgn8[:], 0.0)
                nc.vector.tensor_scalar_mul(
                    nalpha[:].rearrange("p (t f) -> p t f", f=4), ia3[:, :, 1:5], -1.0
                )
                nc.vector.tensor_scalar(
                    oma[:].rearrange("p (t f) -> p t f", f=4), ia3[:, :, 1:5], -1.0,
                    1.0, AluOpType.mult, AluOpType.add,
                )

                # ---- phase 2: row stats + modified rows + transposes ----
                sqq = wp.tile([128, 8 * JD], DT.float32, tag="sqq")
                sqk = wp.tile([128, 8 * JD], DT.float32, tag="sqk")
                nc.scalar.activation(sqq[:], qrow[:], AF.Square)
                nc.scalar.activation(sqk[:], krow[:], AF.Square)
                n2 = wp.tile([128, 64], DT.float32, tag="n2")
                for side, sq_t in ((0, sqq), (1, sqk)):
                    # one 3D-AP reduce over all 32 (t8, h) groups: cols 1..63
                    # of each 64-wide head block
                    sq3 = sq_t[:].rearrange("p (g d) -> p g d", d=D)[:, :, 1:D]
                    nc.vector.tensor_reduce(
                        n2[:, side * 32:(side + 1) * 32].unsqueeze(2),
                        sq3, mybir.AxisListType.X, AluOpType.add,
                    )
                # n = max(exp(0.5*ln(n2)), 1e-7)
                lnn = wp.tile([128, 64], DT.float32, tag="lnn")
                nc.scalar.activation(lnn[:], n2[:], AF.Ln)
                nrm = wp.tile([128, 64], DT.float32, tag="nrm")
                nc.scalar.activation(nrm[:], lnn[:], AF.Exp, scale=0.5)
                nc.vector.tensor_scalar_max(nrm[:], nrm[:], 1e-7)
                e1 = wp.tile([128, 64], DT.float32, tag="e1")
                e2 = wp.tile([128, 64], DT.float32, tag="e2")
                nc.scalar.activation(e1[:], nrm[:], AF.Exp)
                nc.scalar.activation(e2[:], nrm[:], AF.Exp, scale=-1.0)
                csh = wp.tile([128, 64], DT.float32, tag="csh")
                nc.vector.tensor_add(csh[:], e1[:], e2[:])
                nc.vector.tensor_scalar_mul(csh[:], csh[:], 0.5)
                snh = wp.tile([128, 64], DT.float32, tag="snh")
                nc.vector.tensor_sub(snh[:], e1[:], e2[:])
                rcn = wp.tile([128, 64], DT.float32, tag="rcn")
                nc.vector.reciprocal(rcn[:], nrm[:])
                nc.vector.scalar_tensor_tensor(
                    rat[:], snh[:], 0.5, rcn[:], AluOpType.mult, AluOpType.mult
                )
                rrat = wp.tile([128, 64], DT.float32, tag="rrat")
                nc.vector.reciprocal(rrat[:], rat[:])
                nc.vector.tensor_scalar_mul(ratx2[:], rat[:, 0:32], 2.0)
                nc.vector.tensor_mul(gq[:], csh[:, 0:32], rrat[:, 0:32])
                nc.vector.tensor_copy(coshk[:], csh[:, 32:64])
                nc.vector.tensor_scalar_mul(nrk[:], rat[:, 32:64], -1.0)

                # modified rows: qhat = qrow with col0 := gq; khat = -ratk*krow, col0 := coshk
                # qb2 = (1-alpha)*qrow so the se matmul emits (1-alpha)*se directly
                qhat = wp.tile([128, 8 * JD], DT.float32, tag="qhat")
                khat = wp.tile([128, 8 * JD], DT.float32, tag="khat")
                qb2 = wp.tile([128, 8 * JD], DT.float32, tag="qb2")
                nc.vector.tensor_copy(qhat[:], qrow[:])
                oma3 = oma[:].unsqueeze(2).broadcast_to((128, 32, D))
                nc.vector.tensor_tensor(
                    qb2[:].rearrange("p (g d) -> p g d", d=D), 
                    qrow[:].rearrange("p (g d) -> p g d", d=D), oma3, AluOpType.mult,
                )
                nrk3 = nrk[:].unsqueeze(2).broadcast_to((128, 32, D))
                nc.vector.tensor_tensor(
                    khat[:].rearrange("p (g d) -> p g d", d=D),
                    krow[:].rearrange("p (g d) -> p g d", d=D), nrk3, AluOpType.mult,
                )
                # col0 := gq / coshk, one strided copy each over all 32 (t8,h)
                nc.vector.tensor_copy(
                    qhat[:].rearrange("p (g d) -> p g d", d=D)[:, :, 0:1],
                    gq[:].unsqueeze(2),
                )
                nc.vector.tensor_copy(
                    khat[:].rearrange("p (g d) -> p g d", d=D)[:, :, 0:1],
                    coshk[:].unsqueeze(2),
                )

                # transposes: row layout [t, c'] -> column layout [c', t]
                # per-128 blocks rotating through the pj psum tag; evict
                # copies alternate scalar/vector to split the load
                tcnt = 0
                for jc in range(2):
                    for src, dsts, dt_, ident in (
                        (qb2, qbT, DT.bfloat16, idf),
                        (krow, kbT, DT.bfloat16, idf),
                        (qhat, qhT, DT.float32, idf),
                        (khat, khT, DT.float32, idf),
                    ):
                        for t8 in range(8):
                            pst = ps1.tile([128, JQ], DT.float32, tag="pj",
                                           bufs=8, name=f"pt{tcnt}")
                            nc.tensor.transpose(
                                pst[:, 0:128],
                                src[:, t8 * JD + jc * 128: t8 * JD + (jc + 1) * 128],
                                ident[:],
                            )
                            dslice = dsts[jc][:, t8 * 128:(t8 + 1) * 128]
                            if tcnt % 4 == 3:
                                nc.vector.tensor_copy(dslice, pst[:, 0:128])
                            else:
                                nc.scalar.copy(dslice, pst[:, 0:128])
                            tcnt += 1

            # ---- phase 3: attention, software-pipelined emission ----
            # Units are (ti, h); stages are skewed so each engine's in-order
            # queue interleaves ops from different units.
            # Chain per unit over [128, S]:
            #   se matmul -> A = se + czm (DVE, evicts PSUM, applies causal mask)
            #   ni matmul -> d = Ln(ni, scale=2*rat_t) (scalar, evicts PSUM)
            #     [arccosh(z) = ln(2z) exactly to ~5e-5 in d^2 since z >= ~230]
            #   nad = (d * -alpha) * d (gpsimd stt)
            #   A += nad (DVE); pb0 = Exp(SQD*A) accum den (scalar)
            #   pbt = pb0 * spike/den; DMA-transpose; PV + out-proj matmuls.
            with tc.tile_pool(name="pipe", bufs=1) as pp, \
                 tc.tile_pool(name="pipeb", bufs=8) as pb_pool, \
                 tc.tile_pool(name="pipeo", bufs=2) as po, \
                 tc.tile_pool(name="pipes", bufs=6) as sp, \
                 tc.tile_pool(name="psA", bufs=2, space="PSUM") as psA, \
                 tc.tile_pool(name="psY", bufs=2, space="PSUM") as psY:
                UNITS = [(ti, h) for ti in (7, 0, 6, 1, 5, 2, 4, 3) for h in range(HL)]
                NU = len(UNITS)
                st = {}   # per-unit live tiles
                psy_t = {}
                rs_done = []   # tis whose partial_d rows are written
                rs_sent = []   # tis whose reduce-scatter chunk is dispatched

                def stage0(u):
                    ti, h = UNITS[u]
                    S = (ti + 1) * 128
                    jc, hh = h // 2, h % 2
                    A = pp.tile([128, 1024], DT.float32, tag="A", bufs=8, name=f"A{u}")
                    zoff = 1024 - ti * 128
                    for c0 in range(0, S, 512):
                        n_sc = min(512, S - c0)
                        se = psA.tile([128, 512], DT.float32, tag="sc", bufs=2,
                                      name=f"se{u}_{c0}")
                        nc.tensor.matmul(
                            se[:, 0:n_sc],
                            qbT[jc][hh * 64:(hh + 1) * 64, ti * 128:(ti + 1) * 128],
                            kbT[jc][hh * 64:(hh + 1) * 64, c0:c0 + n_sc],
                            start=True, stop=True,
                        )
                        nc.vector.tensor_add(
                            A[:, c0:c0 + n_sc], se[:, 0:n_sc],
                            czm[:, zoff + c0: zoff + c0 + n_sc],
                        )
                    # ni in 512-col psum chunks (4 bufs of 1 bank) so the PE
                    # never stalls on the scalar Ln eviction
                    nis = []
                    for c0 in range(0, S, 512):
                        n_sc = min(512, S - c0)
                        ni = psA.tile([128, 512], DT.float32, tag="ni", bufs=4,
                                      name=f"ni{u}_{c0}")
                        nc.tensor.matmul(
                            ni[:, 0:n_sc],
                            qhT[jc][hh * 64:(hh + 1) * 64, ti * 128:(ti + 1) * 128],
                            khT[jc][hh * 64:(hh + 1) * 64, c0:c0 + n_sc],
                            start=True, stop=True,
                        )
                        nis.append((c0, n_sc, ni))
                    st[u] = (A, nis)

                def stage1(u):
                    ti, h = UNITS[u]
                    col = ti * 4 + h
                    A, nis = st[u]
                    d_t = pp.tile([128, 1024], DT.float32, tag="d", bufs=6,
                                  name=f"d{u}")
                    for c0, n_sc, ni in nis:
                        nc.scalar.activation(d_t[:, c0:c0 + n_sc], ni[:, 0:n_sc],
                                             AF.Ln, scale=ratx2[:, col:col + 1])
                    st[u] = (A, d_t)

                def stage2(u):
                    ti, h = UNITS[u]
                    S = (ti + 1) * 128
                    col = ti * 4 + h
                    A, d_t = st[u]
                    nad = pp.tile([128, 1024], DT.float32, tag="nad", bufs=5,
                                  name=f"nad{u}")
                    nc.gpsimd.tensor_mul(nad[:, :S], d_t[:, :S], d_t[:, :S])
                    st[u] = (A, nad)

                def stage3(u):
                    ti, h = UNITS[u]
                    S = (ti + 1) * 128
                    col = ti * 4 + h
                    A, nad = st.pop(u)
                    nc.vector.scalar_tensor_tensor(
                        A[:, :S], nad[:, :S], nalpha[:, col:col + 1], A[:, :S],
                        AluOpType.mult, AluOpType.add,
                    )
                    den = sp.tile([128, 1], DT.float32, tag="den")
                    pb0 = pb_pool.tile([128, 1024], DT.bfloat16, tag="pb0")
                    nc.scalar.activation(pb0[:, :S], A[:, :S], AF.Exp, scale=SQD,
                                         accum_out=den[:])
                    rec = sp.tile([128, 1], DT.float32, tag="rec")
                    nc.vector.reciprocal(rec[:], den[:])
                    sc2 = sp.tile([128, 1], DT.float32, tag="sc2")
                    nc.vector.tensor_mul(sc2[:], rec[:], spike[:, ti:ti + 1])
                    pbt = pb_pool.tile([128, 1024], DT.bfloat16, tag="pbt")
                    nc.gpsimd.tensor_tensor(
                        pbt[:, :S], pb0[:, :S],
                        sc2[:].broadcast_to((128, S)), AluOpType.mult,
                    )
                    pT = pb_pool.tile([128, 1024], DT.bfloat16, tag="pT")
                    nc.sync.dma_start_transpose(
                        pT[:, :S].rearrange("p (b c) -> p b c", c=128),
                        pbt[:, :S],
                    )
                    st[u] = pT

                def send_rs(p):
                    # per-pair bf16 reduce-scatter chunk: [256, C] -> [64, C]
                    nc.gpsimd.collective_compute(
                        "ReduceScatter", mybir.AluOpType.add,
                        replica_groups=GROUPS,
                        ins=[partial_ds[p][:]],
                        outs=[rs_out_ds[p][:]],
                    )
                    rs_sent.append(p)

                def stage4(u):
                    ti, h = UNITS[u]
                    S = (ti + 1) * 128
                    jc, hh = h // 2, h % 2
                    pT = st.pop(u)
                    if h == 0:
                        psy_t[ti] = psY.tile([128, 2 * 128], DT.float32, tag="psy",
                                             name=f"psy{ti}")
                    psy = psy_t[ti]
                    for sj in range(ti + 1):
                        nc.tensor.matmul(
                            psy[hh * 64:(hh + 1) * 64, jc * 128:(jc + 1) * 128],
                            vbf[:, sj * JD + h * D: sj * JD + (h + 1) * D],
                            pT[:, sj * 128:(sj + 1) * 128],
                            start=(sj == 0), stop=(sj == ti),
                            tile_position=(0, hh * 64),
                        )
                    if h == HL - 1:
                        psy = psy_t.pop(ti)
                        yT0 = sp.tile([128, 128], DT.bfloat16, tag="yT0")
                        yT1 = sp.tile([128, 128], DT.bfloat16, tag="yT1")
                        nc.scalar.copy(yT0[:], psy[:, 0:128])
                        nc.scalar.copy(yT1[:], psy[:, 128:256])
                        out_sb = po.tile([128, 1024], DT.bfloat16, tag="outsb")
                        for oc in range(2):
                            pso = psA.tile([128, 512], DT.float32, tag="sc", bufs=2,
                                           name=f"pso{ti}_{oc}")
                            for cc, yT_t in ((0, yT0), (1, yT1)):
                                nc.tensor.matmul(
                                    pso[:, 0:512],
                                    yT_t[:],
                                    wobf[:, cc * C + oc * 512: cc * C + oc * 512 + 512],
                                    start=(cc == 0), stop=(cc == 1),
                                )
                            nc.scalar.copy(
                                out_sb[:, oc * 512:(oc + 1) * 512], pso[:, 0:512]
                            )
                        p, slot = next(
                            (pp, sl) for pp, pr in enumerate(RS_PAIRS)
                            for sl, t in enumerate(pr) if t == ti
                        )
                        nc.sync.dma_start(
                            out=partial_ds[p][slot * 128:(slot + 1) * 128, :],
                            in_=out_sb[:],
                        )
                        if slot == 1:
                            rs_done.append(p)

                # two units per pipeline step: denser per-engine bursts keep
                # the PE past the HAM activity window (warm clock) and
                # amortize semaphore hops
                for step in range(NU // 2 + 5):
                    for par in range(2):
                        u = step * 2 + par
                        if u < NU:
                            stage0(u)
                    for par in range(2):
                        u = (step - 1) * 2 + par
                        if 0 <= u < NU:
                            stage1(u)
                    for par in range(2):
                        u = (step - 2) * 2 + par
                        if 0 <= u < NU:
                            stage2(u)
                    for par in range(2):
                        u = (step - 3) * 2 + par
                        if 0 <= u < NU:
                            stage3(u)
                    for par in range(2):
                        u = (step - 4) * 2 + par
                        if 0 <= u < NU:
                            stage4(u)
                    # dispatch each RS chunk as soon as its pair completes
                    while len(rs_sent) < len(rs_done):
                        send_rs(rs_done[len(rs_sent)])

                while len(rs_sent) < len(rs_done):
                    send_rs(rs_done[len(rs_sent)])

                # ---- per-chunk final: add bout, store fp32 ----
                # out_e rows: chunk p covers pair (ta, tb): rows [p*64, p*64+64)
                # map to (ti, r-strip) in _assemble.
                for p in range(4):
                    finb = po.tile([128, 1024], DT.bfloat16, tag="finb")
                    nc.sync.dma_start(out=finb[0:64, :], in_=rs_out_ds[p][:])
                    fin = po.tile([128, 1024], DT.float32, tag="fin")
                    nc.vector.tensor_add(fin[0:64, :], finb[0:64, :], bout_b[0:64, :])
                    nc.sync.dma_start(out=out_e[p * 64:(p + 1) * 64, :],
                                      in_=fin[0:64, :])

    nc.finalize()
    return nc


_NC = None


def _get_nc():
    global _NC
    if _NC is None:
        _NC = build_nc()
    return _NC


def _shard_inputs(inputs):
    x = np.asarray(inputs["x"], np.float32)
    Wqkv = np.asarray(inputs["Wqkv"], np.float32)
    bqkv = np.asarray(inputs["bqkv"], np.float32)
    Wout = np.asarray(inputs["Wout"], np.float32)
    bout = np.asarray(inputs["bout"], np.float32)
    Wimp = np.asarray(inputs["Wimp"], np.float32)
    bimp = np.asarray(inputs["bimp"], np.float32)
    Walpha = np.asarray(inputs["Walpha"], np.float32)
    balpha = np.asarray(inputs["balpha"], np.float32)
    th = np.asarray(inputs["threshold"], np.float32)

    import ml_dtypes
    cmask = np.triu(np.full((128, 128), NEG, np.float32), 1)
    in_maps = []
    for core in range(N_CORES):
        b = core // 4
        hs = (core % 4) * HL
        sl = slice(hs * D, (hs + HL) * D)
        m = {
            "xT": np.ascontiguousarray(x[b].T),
            "wqiT": np.ascontiguousarray(np.concatenate(
                [Wqkv[sl], Wimp, Walpha[hs:hs + HL],
                 np.zeros((1, C), np.float32)], 0).T),
            "wkT": np.ascontiguousarray(Wqkv[C + hs * D: C + (hs + HL) * D].T),
            "wvT": np.ascontiguousarray(Wqkv[2 * C + hs * D: 2 * C + (hs + HL) * D].T),
            "bq_b": np.ascontiguousarray(np.broadcast_to(bqkv[sl], (128, JD))),
            "bk_b": np.ascontiguousarray(
                np.broadcast_to(bqkv[C + hs * D: C + (hs + HL) * D], (128, JD))),
            "bv_b": np.ascontiguousarray(
                np.broadcast_to(bqkv[2 * C + hs * D: 2 * C + (hs + HL) * D], (128, JD))),
            "bia_b": np.ascontiguousarray(np.broadcast_to(
                np.concatenate([bimp, balpha[hs:hs + HL]]), (128, 5))),
            "woT": np.ascontiguousarray(
                Wout[:, sl].T.astype(ml_dtypes.bfloat16)),
            "bout_b": np.ascontiguousarray(np.broadcast_to(bout, (128, C))),
            "thneg_b": np.full((128, 1), -th[0], np.float32),
            "cmask": cmask,
        }
        in_maps.append(m)
    return in_maps


def kernel(**inputs):
    nc = _get_nc()
    in_maps = _shard_inputs(inputs)
    trace = os.environ.get("KERNEL_PROFILE", "") == "1"
    res = run_bass_kernel_spmd(
        nc, in_maps, core_ids=list(range(N_CORES)), trace=trace
    )
    KSTATS["exec_time_ns"] = res.exec_time_ns
    return _assemble({c: res.results[c] for c in range(N_CORES)})


RS_PAIRS_HOST = [(7, 0), (6, 1), (5, 2), (4, 3)]


def _assemble(results):
    # pair-chunked reduce-scatter: chunk p holds [ti_a(128 rows) | ti_b(128)];
    # RS gives rank r the contiguous 64-row strip r of that 256-row chunk.
    out = np.zeros((B, T, C), np.float32)
    for core in range(N_CORES):
        b, r = core // 4, core % 4
        res = results[core]["out"]
        for p, (ta, tb) in enumerate(RS_PAIRS_HOST):
            ti = ta if r < 2 else tb
            off = (r % 2) * 64
            out[b, ti * 128 + off: ti * 128 + off + 64, :] = \
                res[p * 64:(p + 1) * 64, :]
    return out



# revision 50
# speedup vs baseline: 1.1081x; 1.1081x over previous
"""AdaptiveGeometryAttention distributed Bass kernel for 8 trn2 NeuronCores.

Sharding: data-parallel over B (2 groups of 4 cores), head-parallel over H
(4 heads per core). Each core computes its heads' attention and a partial
out-projection [T, C]; a ReduceScatter(add) over each 4-core group leaves
each core with a distinct 256-row shard of the final output, which the host
reassembles.

Self-contained: hardcodes all shapes; host side only shards/transposes
inputs and concatenates the output shards.
"""
import os
import sys

for _p in ("/opt/trn_rl_repo",):
    if _p not in sys.path:
        sys.path.append(_p)

import numpy as np
import concourse.bass as bass
import concourse.bacc as bacc
import concourse.mybir as mybir
from concourse import masks
from concourse.alu_op_type import AluOpType
from concourse.tile import TileContext
from concourse.bass_utils import run_bass_kernel_spmd

AF = mybir.ActivationFunctionType
DT = mybir.dt

B, T, C, H, D = 2, 1024, 1024, 16, 64
HL = 4                 # heads per core
JD = HL * D            # 256 local head dims
N_CORES = 8
GROUPS = [[0, 1, 2, 3], [4, 5, 6, 7]]
SQD = 0.125            # 1/sqrt(D)
NEG = -1.0e30

# dtype knobs
PROJ_F32R = True       # q/k/v/ia projection matmuls via float32r operands
NI_F32R = True         # neg_inner matmul via float32r operands

KSTATS = {}

# The act-table-load placement pass picks the FIRST set containing each
# activation function, so alternating Ln/Exp thrashes between the
# single-function sets (~50 table reloads per kernel). Strip ln/exp from
# those sets so the combined natural_log_exp_and_others set is chosen.
_orig_get_tables = bacc.get_activation_tables


def _patched_get_tables(arch):
    t = _orig_get_tables(arch)
    for nm in ("exp_and_others", "natural_log", "exp_and_friends"):
        if nm in t:
            t[nm] = t[nm] - {AF.Exp, AF.Ln}
    return t


bacc.get_activation_tables = _patched_get_tables


def _f32r(ap):
    return ap.bitcast(DT.float32r)


def _mmdt(ap, use_f32r):
    return _f32r(ap) if use_f32r else ap


def build_nc():
    nc = bacc.Bacc("TRN2")

    # ---- I/O ----
    DT_PROJ = DT.float32r if PROJ_F32R else DT.float32
    DT_NI = DT.float32r if NI_F32R else DT.float32
    JQ = JD + 6            # q proj cols + [imp, alpha*4, pad] cols
    xT_e = nc.dram_tensor("xT", [C, T], DT_PROJ, kind="ExternalInput")
    wqiT_e = nc.dram_tensor("wqiT", [C, JQ], DT_PROJ, kind="ExternalInput")
    wkT_e = nc.dram_tensor("wkT", [C, JD], DT_PROJ, kind="ExternalInput")
    wvT_e = nc.dram_tensor("wvT", [C, JD], DT_PROJ, kind="ExternalInput")
    bq_e = nc.dram_tensor("bq_b", [128, JD], DT.float32, kind="ExternalInput")
    bk_e = nc.dram_tensor("bk_b", [128, JD], DT.float32, kind="ExternalInput")
    bv_e = nc.dram_tensor("bv_b", [128, JD], DT.float32, kind="ExternalInput")
    bia_e = nc.dram_tensor("bia_b", [128, 5], DT.float32, kind="ExternalInput")
    woT_e = nc.dram_tensor("woT", [JD, C], DT.bfloat16, kind="ExternalInput")
    bout_e = nc.dram_tensor("bout_b", [128, C], DT.float32, kind="ExternalInput")
    thneg_e = nc.dram_tensor("thneg_b", [128, 1], DT.float32, kind="ExternalInput")
    cmask_e = nc.dram_tensor("cmask", [128, 128], DT.float32, kind="ExternalInput")
    out_e = nc.dram_tensor("out", [T // 4, C], DT.float32, kind="ExternalOutput")

    # per-chunk staging: separate dram tensors so Tile's per-tensor dependency
    # tracking lets chunk k+1's partial write overlap chunk k's collective
    # (a single tensor serializes: write-after-read on the whole tensor).
    # Chunks pair tis by completion order (7,0),(6,1),(5,2),(4,3).
    RS_PAIRS = [(7, 0), (6, 1), (5, 2), (4, 3)]
    partial_ds = [nc.dram_tensor(f"partial{p}", [256, C], DT.bfloat16)
                  for p in range(4)]
    rs_out_ds = [nc.dram_tensor(f"rsout{p}", [64, C], DT.bfloat16)
                 for p in range(4)]
    # tiny warm-up collective: absorbs inter-core launch skew so the real
    # reduce-scatters don't each pay a peer-wait mid-pipeline
    warm_in_d = nc.dram_tensor("warm_in", [4, 128], DT.float32)
    warm_out_d = nc.dram_tensor("warm_out", [1, 128], DT.float32)

    with TileContext(nc) as tc:
        # The scheduler serializes every DMA-transpose issued after a
        # collective against that collective's completion, which stalls the
        # attention pipeline (~15us per reduce-scatter chunk). Our transposes
        # (SBUF->SBUF, core DMA queues) are independent of the HBM->HBM
        # reduce-scatter chunks, so drop that edge.
        class _NoAddSet(set):
            def add(self, _):
                pass

        tc.serialize_transpose_collective_names = _NoAddSet()
        with (
            tc.tile_pool(name="const", bufs=1) as cpool,
            tc.tile_pool(name="mainp", bufs=1) as mp,
        ):
            # ---- warm-up collective (see warm_in_d) ----
            nc.gpsimd.collective_compute(
                "ReduceScatter", mybir.AluOpType.add,
                replica_groups=GROUPS,
                ins=[warm_in_d[:]],
                outs=[warm_out_d[:]],
            )
            # ---- constants ----
            idf = cpool.tile([128, 128], DT.float32, tag="idf")
            masks.make_identity(nc, idf[:])
            idbf = cpool.tile([128, 128], DT.bfloat16, tag="idbf")
            masks.make_identity(nc, idbf[:])
            cmask = cpool.tile([128, 128], DT.float32, tag="cmask")
            nc.sync.dma_start(out=cmask[:], in_=cmask_e[:])
            czm = cpool.tile([128, 1152], DT.float32, tag="czm")
            nc.vector.memset(czm[:, 0:1024], 0.0)
            nc.vector.tensor_copy(czm[:, 1024:1152], cmask[:])
            bout_b = cpool.tile([128, C], DT.float32, tag="boutb")
            thneg = cpool.tile([128, 1], DT.float32, tag="thneg")
            nc.sync.dma_start(out=thneg[:], in_=thneg_e[:])

            # ---- persistent main tiles ----
            vbf = mp.tile([128, 8 * JD], DT.bfloat16, tag="vbf")
            qbT = [mp.tile([128, T], DT.bfloat16, tag=f"qbT{j}", name=f"qbT{j}") for j in range(2)]
            kbT = [mp.tile([128, T], DT.bfloat16, tag=f"kbT{j}", name=f"kbT{j}") for j in range(2)]
            qhT = [mp.tile([128, T], DT_NI, tag=f"qhT{j}", name=f"qhT{j}") for j in range(2)]
            khT = [mp.tile([128, T], DT_NI, tag=f"khT{j}", name=f"khT{j}") for j in range(2)]
            wobf = mp.tile([128, 2 * C], DT.bfloat16, tag="wobf")
            nalpha = mp.tile([128, 32], DT.float32, tag="nalpha")
            oma = mp.tile([128, 32], DT.float32, tag="oma")
            spike = mp.tile([128, 8], DT.float32, tag="spike")
            # row stats, col = side*32 + ti*4 + h
            rat = mp.tile([128, 64], DT.float32, tag="rat")    # sinh(n)/n
            ratx2 = mp.tile([128, 32], DT.float32, tag="ratx2")  # 2*rat (q side)
            gq = mp.tile([128, 32], DT.float32, tag="gq")      # cosh/ratio (q side)
            coshk = mp.tile([128, 32], DT.float32, tag="coshk")
            nrk = mp.tile([128, 32], DT.float32, tag="nrk")    # -ratio_k

            with tc.tile_pool(name="wpool", bufs=1) as wp, \
                 tc.tile_pool(name="ps1", bufs=8, space="PSUM") as ps1:
                # ---- loads: interleave x and weight chunks so the kc-outer
                # projection matmuls can start as soon as chunk 0 lands ----
                xT = wp.tile([128, 8 * T], DT_PROJ, tag="xT")
                wqi = wp.tile([128, 8 * JQ], DT_PROJ, tag="wqi")
                wk = wp.tile([128, 8 * JD], DT_PROJ, tag="wk")
                wv = wp.tile([128, 8 * JD], DT_PROJ, tag="wv")
                for kc in range(8):
                    nc.sync.dma_start(
                        out=xT[:, kc * T:(kc + 1) * T],
                        in_=xT_e[kc * 128:(kc + 1) * 128, :],
                    )
                    nc.sync.dma_start(
                        out=wqi[:, kc * JQ:(kc + 1) * JQ],
                        in_=wqiT_e[kc * 128:(kc + 1) * 128, :],
                    )
                    nc.sync.dma_start(
                        out=wk[:, kc * JD:(kc + 1) * JD],
                        in_=wkT_e[kc * 128:(kc + 1) * 128, :],
                    )
                    nc.sync.dma_start(
                        out=wv[:, kc * JD:(kc + 1) * JD],
                        in_=wvT_e[kc * 128:(kc + 1) * 128, :],
                    )
                bq_b = wp.tile([128, JD], DT.float32, tag="bqb")
                bk_b = wp.tile([128, JD], DT.float32, tag="bkb")
                bv_b = wp.tile([128, JD], DT.float32, tag="bvb")
                bia_b = wp.tile([128, 5], DT.float32, tag="biab")
                nc.sync.dma_start(out=bq_b[:], in_=bq_e[:])
                nc.sync.dma_start(out=bk_b[:], in_=bk_e[:])
                nc.sync.dma_start(out=bv_b[:], in_=bv_e[:])
                nc.sync.dma_start(out=bia_b[:], in_=bia_e[:])
                for cc in range(2):
                    nc.sync.dma_start(
                        out=wobf[:, cc * C:(cc + 1) * C],
                        in_=woT_e[cc * 128:(cc + 1) * 128, :],
                    )
                nc.sync.dma_start(out=bout_b[:], in_=bout_e[:])

                # ---- phase 1: projections, kc-outer over 8 live psum tiles
                # (row layout [t, d']); ia cols ride along with q ----
                qrow = wp.tile([128, 8 * JD], DT.float32, tag="qrow")
                krow = wp.tile([128, 8 * JD], DT.float32, tag="krow")
                ia_sb = wp.tile([128, 8 * 5], DT.float32, tag="iasb")

                for pnm, (dst, w_t, b_t, jw) in enumerate((
                    (qrow, wqi, bq_b, JQ),
                    (krow, wk, bk_b, JD),
                    (vbf, wv, bv_b, JD),
                )):
                    pj = [ps1.tile([128, JQ], DT.float32, tag="pj", bufs=8,
                                   name=f"pj{pnm}_{t8}")
                          for t8 in range(8)]
                    for kc in range(8):
                        for t8 in range(8):
                            nc.tensor.matmul(
                                pj[t8][:, 0:jw],
                                xT[:, kc * T + t8 * 128: kc * T + t8 * 128 + 128],
                                w_t[:, kc * jw:(kc + 1) * jw],
                                start=(kc == 0), stop=(kc == 7),
                            )
                    for t8 in range(8):
                        nc.vector.tensor_add(
                            dst[:, t8 * JD:(t8 + 1) * JD], pj[t8][:, 0:JD], b_t[:]
                        )
                        if jw == JQ:
                            tmp5 = wp.tile([128, 5], DT.float32, tag="tmp5")
                            nc.vector.tensor_add(tmp5[:], pj[t8][:, JD:JD + 5],
                                                 bia_b[:])
                            nc.scalar.activation(ia_sb[:, t8 * 5:(t8 + 1) * 5],
                                                 tmp5[:], AF.Sigmoid)

                # ---- spike / nalpha ----
                ia3 = ia_sb[:].rearrange("p (t f) -> p t f", f=5)
                # importance[t=0] := 0 (cmask) before threshold compare
                nc.vector.memset(ia_sb[0:1, 0:1], 0.0)
                imp8 = wp.tile([128, 8], DT.float32, tag="imp8")
                nc.vector.tensor_scalar(imp8[:], ia3[:, :, 0:1], thneg[:], None, AluOpType.add)
                sgn8 = wp.tile([128, 8], DT.float32, tag="sgn8")
                nc.scalar.activation(sgn8[:], imp8[:], AF.Sign)
                nc.vector.tensor_scalar_max(spike[:], sgn8[:], 0.0)
                nc.vector.tensor_scalar_mul(
                    nalpha[:].rearrange("p (t f) -> p t f", f=4), ia3[:, :, 1:5], -1.0
                )
                nc.vector.tensor_scalar(
                    oma[:].rearrange("p (t f) -> p t f", f=4), ia3[:, :, 1:5], -1.0,
                    1.0, AluOpType.mult, AluOpType.add,
                )

                # ---- phase 2: row stats + modified rows + transposes ----
                sqq = wp.tile([128, 8 * JD], DT.float32, tag="sqq")
                sqk = wp.tile([128, 8 * JD], DT.float32, tag="sqk")
                nc.scalar.activation(sqq[:], qrow[:], AF.Square)
                nc.scalar.activation(sqk[:], krow[:], AF.Square)
                n2 = wp.tile([128, 64], DT.float32, tag="n2")
                for side, sq_t in ((0, sqq), (1, sqk)):
                    # one 3D-AP reduce over all 32 (t8, h) groups: cols 1..63
                    # of each 64-wide head block
                    sq3 = sq_t[:].rearrange("p (g d) -> p g d", d=D)[:, :, 1:D]
                    nc.vector.tensor_reduce(
                        n2[:, side * 32:(side + 1) * 32].unsqueeze(2),
                        sq3, mybir.AxisListType.X, AluOpType.add,
                    )
                # n = max(exp(0.5*ln(n2)), 1e-7)
                lnn = wp.tile([128, 64], DT.float32, tag="lnn")
                nc.scalar.activation(lnn[:], n2[:], AF.Ln)
                nrm = wp.tile([128, 64], DT.float32, tag="nrm")
                nc.scalar.activation(nrm[:], lnn[:], AF.Exp, scale=0.5)
                nc.vector.tensor_scalar_max(nrm[:], nrm[:], 1e-7)
                e1 = wp.tile([128, 64], DT.float32, tag="e1")
                e2 = wp.tile([128, 64], DT.float32, tag="e2")
                nc.scalar.activation(e1[:], nrm[:], AF.Exp)
                nc.scalar.activation(e2[:], nrm[:], AF.Exp, scale=-1.0)
                csh = wp.tile([128, 64], DT.float32, tag="csh")
                nc.vector.tensor_add(csh[:], e1[:], e2[:])
                nc.vector.tensor_scalar_mul(csh[:], csh[:], 0.5)
                snh = wp.tile([128, 64], DT.float32, tag="snh")
                nc.vector.tensor_sub(snh[:], e1[:], e2[:])
                rcn = wp.tile([128, 64], DT.float32, tag="rcn")
                nc.vector.reciprocal(rcn[:], nrm[:])
                nc.vector.scalar_tensor_tensor(
                    rat[:], snh[:], 0.5, rcn[:], AluOpType.mult, AluOpType.mult
                )
                rrat = wp.tile([128, 64], DT.float32, tag="rrat")
                nc.vector.reciprocal(rrat[:], rat[:])
                nc.vector.tensor_scalar_mul(ratx2[:], rat[:, 0:32], 2.0)
                nc.vector.tensor_mul(gq[:], csh[:, 0:32], rrat[:, 0:32])
                nc.vector.tensor_copy(coshk[:], csh[:, 32:64])
                nc.vector.tensor_scalar_mul(nrk[:], rat[:, 32:64], -1.0)

                # modified rows: qhat = qrow with col0 := gq; khat = -ratk*krow, col0 := coshk
                # qb2 = (1-alpha)*qrow so the se matmul emits (1-alpha)*se directly
                qhat = wp.tile([128, 8 * JD], DT.float32, tag="qhat")
                khat = wp.tile([128, 8 * JD], DT.float32, tag="khat")
                qb2 = wp.tile([128, 8 * JD], DT.float32, tag="qb2")
                nc.vector.tensor_copy(qhat[:], qrow[:])
                oma3 = oma[:].unsqueeze(2).broadcast_to((128, 32, D))
                nc.vector.tensor_tensor(
                    qb2[:].rearrange("p (g d) -> p g d", d=D), 
                    qrow[:].rearrange("p (g d) -> p g d", d=D), oma3, AluOpType.mult,
                )
                nrk3 = nrk[:].unsqueeze(2).broadcast_to((128, 32, D))
                nc.vector.tensor_tensor(
                    khat[:].rearrange("p (g d) -> p g d", d=D),
                    krow[:].rearrange("p (g d) -> p g d", d=D), nrk3, AluOpType.mult,
                )
                # col0 := gq / coshk, one strided copy each over all 32 (t8,h)
                nc.vector.tensor_copy(
                    qhat[:].rearrange("p (g d) -> p g d", d=D)[:, :, 0:1],
                    gq[:].unsqueeze(2),
                )
                nc.vector.tensor_copy(
                    khat[:].rearrange("p (g d) -> p g d", d=D)[:, :, 0:1],
                    coshk[:].unsqueeze(2),
                )

                # transposes: row layout [t, c'] -> column layout [c', t]
                # per-128 blocks rotating through the pj psum tag; evict
                # copies alternate scalar/vector to split the load
                tcnt = 0
                for jc in range(2):
                    for src, dsts, dt_, ident in (
                        (qb2, qbT, DT.bfloat16, idf),
                        (krow, kbT, DT.bfloat16, idf),
                        (qhat, qhT, DT.float32, idf),
                        (khat, khT, DT.float32, idf),
                    ):
                        for t8 in range(8):
                            pst = ps1.tile([128, JQ], DT.float32, tag="pj",
                                           bufs=8, name=f"pt{tcnt}")
                            nc.tensor.transpose(
                                pst[:, 0:128],
                                src[:, t8 * JD + jc * 128: t8 * JD + (jc + 1) * 128],
                                ident[:],
                            )
                            dslice = dsts[jc][:, t8 * 128:(t8 + 1) * 128]
                            if tcnt % 4 == 3:
                                nc.vector.tensor_copy(dslice, pst[:, 0:128])
                            else:
                                nc.scalar.copy(dslice, pst[:, 0:128])
                            tcnt += 1

            # ---- phase 3: attention, software-pipelined emission ----
            # Units are (ti, h); stages are skewed so each engine's in-order
            # queue interleaves ops from different units.
            # Chain per unit over [128, S]:
            #   se matmul -> A = se + czm (DVE, evicts PSUM, applies causal mask)
            #   ni matmul -> d = Ln(ni, scale=2*rat_t) (scalar, evicts PSUM)
            #     [arccosh(z) = ln(2z) exactly to ~5e-5 in d^2 since z >= ~230]
            #   nad = (d * -alpha) * d (gpsimd stt)
            #   A += nad (DVE); pb0 = Exp(SQD*A) accum den (scalar)
            #   pbt = pb0 * spike/den; DMA-transpose; PV + out-proj matmuls.
            with tc.tile_pool(name="pipe", bufs=1) as pp, \
                 tc.tile_pool(name="pipeb", bufs=8) as pb_pool, \
                 tc.tile_pool(name="pipeo", bufs=2) as po, \
                 tc.tile_pool(name="pipes", bufs=6) as sp, \
                 tc.tile_pool(name="psA", bufs=2, space="PSUM") as psA, \
                 tc.tile_pool(name="psY", bufs=2, space="PSUM") as psY:
                UNITS = [(ti, h) for ti in (7, 0, 6, 1, 5, 2, 4, 3) for h in range(HL)]
                NU = len(UNITS)
                st = {}   # per-unit live tiles
                psy_t = {}
                rs_done = []   # tis whose partial_d rows are written
                rs_sent = []   # tis whose reduce-scatter chunk is dispatched

                def stage0(u):
                    ti, h = UNITS[u]
                    S = (ti + 1) * 128
                    jc, hh = h // 2, h % 2
                    A = pp.tile([128, 1024], DT.float32, tag="A", bufs=8, name=f"A{u}")
                    zoff = 1024 - ti * 128
                    for c0 in range(0, S, 512):
                        n_sc = min(512, S - c0)
                        se = psA.tile([128, 512], DT.float32, tag="sc", bufs=2,
                                      name=f"se{u}_{c0}")
                        nc.tensor.matmul(
                            se[:, 0:n_sc],
                            qbT[jc][hh * 64:(hh + 1) * 64, ti * 128:(ti + 1) * 128],
                            kbT[jc][hh * 64:(hh + 1) * 64, c0:c0 + n_sc],
                            start=True, stop=True,
                        )
                        nc.vector.tensor_add(
                            A[:, c0:c0 + n_sc], se[:, 0:n_sc],
                            czm[:, zoff + c0: zoff + c0 + n_sc],
                        )
                    # ni in 512-col psum chunks (4 bufs of 1 bank) so the PE
                    # never stalls on the scalar Ln eviction
                    nis = []
                    for c0 in range(0, S, 512):
                        n_sc = min(512, S - c0)
                        ni = psA.tile([128, 512], DT.float32, tag="ni", bufs=4,
                                      name=f"ni{u}_{c0}")
                        nc.tensor.matmul(
                            ni[:, 0:n_sc],
                            qhT[jc][hh * 64:(hh + 1) * 64, ti * 128:(ti + 1) * 128],
                            khT[jc][hh * 64:(hh + 1) * 64, c0:c0 + n_sc],
                            start=True, stop=True,
                        )
                        nis.append((c0, n_sc, ni))
                    st[u] = (A, nis)

                def stage1(u):
                    ti, h = UNITS[u]
                    col = ti * 4 + h
                    A, nis = st[u]
                    d_t = pp.tile([128, 1024], DT.float32, tag="d", bufs=6,
                                  name=f"d{u}")
                    for c0, n_sc, ni in nis:
                        nc.scalar.activation(d_t[:, c0:c0 + n_sc], ni[:, 0:n_sc],
                                             AF.Ln, scale=ratx2[:, col:col + 1])
                    st[u] = (A, d_t)

                def stage2(u):
                    ti, h = UNITS[u]
                    S = (ti + 1) * 128
                    col = ti * 4 + h
                    A, d_t = st[u]
                    nad = pp.tile([128, 1024], DT.float32, tag="nad", bufs=5,
                                  name=f"nad{u}")
                    nc.gpsimd.tensor_mul(nad[:, :S], d_t[:, :S], d_t[:, :S])
                    st[u] = (A, nad)

                def stage3(u):
                    ti, h = UNITS[u]
                    S = (ti + 1) * 128
                    col = ti * 4 + h
                    A, nad = st.pop(u)
                    nc.vector.scalar_tensor_tensor(
                        A[:, :S], nad[:, :S], nalpha[:, col:col + 1], A[:, :S],
                        AluOpType.mult, AluOpType.add,
                    )
                    den = sp.tile([128, 1], DT.float32, tag="den")
                    pb0 = pb_pool.tile([128, 1024], DT.bfloat16, tag="pb0")
                    nc.scalar.activation(pb0[:, :S], A[:, :S], AF.Exp, scale=SQD,
                                         accum_out=den[:])
                    rec = sp.tile([128, 1], DT.float32, tag="rec")
                    nc.vector.reciprocal(rec[:], den[:])
                    sc2 = sp.tile([128, 1], DT.float32, tag="sc2")
                    nc.vector.tensor_mul(sc2[:], rec[:], spike[:, ti:ti + 1])
                    pbt = pb_pool.tile([128, 1024], DT.bfloat16, tag="pbt")
                    nc.gpsimd.tensor_tensor(
                        pbt[:, :S], pb0[:, :S],
                        sc2[:].broadcast_to((128, S)), AluOpType.mult,
                    )
                    pT = pb_pool.tile([128, 1024], DT.bfloat16, tag="pT")
                    nc.sync.dma_start_transpose(
                        pT[:, :S].rearrange("p (b c) -> p b c", c=128),
                        pbt[:, :S],
                    )
                    st[u] = pT

                def send_rs(p):
                    # per-pair bf16 reduce-scatter chunk: [256, C] -> [64, C]
                    nc.gpsimd.collective_compute(
                        "ReduceScatter", mybir.AluOpType.add,
                        replica_groups=GROUPS,
                        ins=[partial_ds[p][:]],
                        outs=[rs_out_ds[p][:]],
                    )
                    rs_sent.append(p)

                def stage4(u):
                    ti, h = UNITS[u]
                    S = (ti + 1) * 128
                    jc, hh = h // 2, h % 2
                    pT = st.pop(u)
                    if h == 0:
                        psy_t[ti] = psY.tile([128, 2 * 128], DT.float32, tag="psy",
                                             name=f"psy{ti}")
                    psy = psy_t[ti]
                    for sj in range(ti + 1):
                        nc.tensor.matmul(
                            psy[hh * 64:(hh + 1) * 64, jc * 128:(jc + 1) * 128],
                            vbf[:, sj * JD + h * D: sj * JD + (h + 1) * D],
                            pT[:, sj * 128:(sj + 1) * 128],
                            start=(sj == 0), stop=(sj == ti),
                            tile_position=(0, hh * 64),
                        )
                    if h == HL - 1:
                        psy = psy_t.pop(ti)
                        yT0 = sp.tile([128, 128], DT.bfloat16, tag="yT0")
                        yT1 = sp.tile([128, 128], DT.bfloat16, tag="yT1")
                        nc.scalar.copy(yT0[:], psy[:, 0:128])
                        nc.scalar.copy(yT1[:], psy[:, 128:256])
                        out_sb = po.tile([128, 1024], DT.bfloat16, tag="outsb")
                        for oc in range(2):
                            pso = psA.tile([128, 512], DT.float32, tag="sc", bufs=2,
                                           name=f"pso{ti}_{oc}")
                            for cc, yT_t in ((0, yT0), (1, yT1)):
                                nc.tensor.matmul(
                                    pso[:, 0:512],
                                    yT_t[:],
                                    wobf[:, cc * C + oc * 512: cc * C + oc * 512 + 512],
                                    start=(cc == 0), stop=(cc == 1),
                                )
                            nc.scalar.copy(
                                out_sb[:, oc * 512:(oc + 1) * 512], pso[:, 0:512]
                            )
                        p, slot = next(
                            (pp, sl) for pp, pr in enumerate(RS_PAIRS)
                            for sl, t in enumerate(pr) if t == ti
                        )
                        nc.sync.dma_start(
                            out=partial_ds[p][slot * 128:(slot + 1) * 128, :],
                            in_=out_sb[:],
                        )
                        if slot == 1:
                            rs_done.append(p)

                # two units per pipeline step: denser per-engine bursts keep
                # the PE past the HAM activity window (warm clock) and
                # amortize semaphore hops
                for step in range(NU // 2 + 5):
                    for par in range(2):
                        u = step * 2 + par
                        if u < NU:
                            stage0(u)
                    for par in range(2):
                        u = (step - 1) * 2 + par
                        if 0 <= u < NU:
                            stage1(u)
                    for par in range(2):
                        u = (step - 2) * 2 + par
                        if 0 <= u < NU:
                            stage2(u)
                    for par in range(2):
                        u = (step - 3) * 2 + par
                        if 0 <= u < NU:
                            stage3(u)
                    for par in range(2):
                        u = (step - 4) * 2 + par
                        if 0 <= u < NU:
                            stage4(u)
                    # dispatch each RS chunk as soon as its pair completes
                    while len(rs_sent) < len(rs_done):
                        send_rs(rs_done[len(rs_sent)])

                while len(rs_sent) < len(rs_done):
                    send_rs(rs_done[len(rs_sent)])

                # ---- per-chunk final: add bout, store fp32 ----
                # out_e rows: chunk p covers pair (ta, tb): rows [p*64, p*64+64)
                # map to (ti, r-strip) in _assemble.
                for p in range(4):
                    finb = po.tile([128, 1024], DT.bfloat16, tag="finb")
                    nc.sync.dma_start(out=finb[0:64, :], in_=rs_out_ds[p][:])
                    fin = po.tile([128, 1024], DT.float32, tag="fin")
                    nc.vector.tensor_add(fin[0:64, :], finb[0:64, :], bout_b[0:64, :])
                    nc.sync.dma_start(out=out_e[p * 64:(p + 1) * 64, :],
                                      in_=fin[0:64, :])

    nc.finalize()
    return nc


_NC = None


def _get_nc():
    global _NC
    if _NC is None:
        _NC = build_nc()
    return _NC


def _shard_inputs(inputs):
    x = np.asarray(inputs["x"], np.float32)
    Wqkv = np.asarray(inputs["Wqkv"], np.float32)
    bqkv = np.asarray(inputs["bqkv"], np.float32)
    Wout = np.asarray(inputs["Wout"], np.float32)
    bout = np.asarray(inputs["bout"], np.float32)
    Wimp = np.asarray(inputs["Wimp"], np.float32)
    bimp = np.asarray(inputs["bimp"], np.float32)
    Walpha = np.asarray(inputs["Walpha"], np.float32)
    balpha = np.asarray(inputs["balpha"], np.float32)
    th = np.asarray(inputs["threshold"], np.float32)

    import ml_dtypes
    cmask = np.triu(np.full((128, 128), NEG, np.float32), 1)
    in_maps = []
    for core in range(N_CORES):
        b = core // 4
        hs = (core % 4) * HL
        sl = slice(hs * D, (hs + HL) * D)
        m = {
            "xT": np.ascontiguousarray(x[b].T),
            "wqiT": np.ascontiguousarray(np.concatenate(
                [Wqkv[sl], Wimp, Walpha[hs:hs + HL],
                 np.zeros((1, C), np.float32)], 0).T),
            "wkT": np.ascontiguousarray(Wqkv[C + hs * D: C + (hs + HL) * D].T),
            "wvT": np.ascontiguousarray(Wqkv[2 * C + hs * D: 2 * C + (hs + HL) * D].T),
            "bq_b": np.ascontiguousarray(np.broadcast_to(bqkv[sl], (128, JD))),
            "bk_b": np.ascontiguousarray(
                np.broadcast_to(bqkv[C + hs * D: C + (hs + HL) * D], (128, JD))),
            "bv_b": np.ascontiguousarray(
                np.broadcast_to(bqkv[2 * C + hs * D: 2 * C + (hs + HL) * D], (128, JD))),
            "bia_b": np.ascontiguousarray(np.broadcast_to(
                np.concatenate([bimp, balpha[hs:hs + HL]]), (128, 5))),
            "woT": np.ascontiguousarray(
                Wout[:, sl].T.astype(ml_dtypes.bfloat16)),
            "bout_b": np.ascontiguousarray(np.broadcast_to(bout, (128, C))),
            "thneg_b": np.full((128, 1), -th[0], np.float32),
            "cmask": cmask,
        }
        in_maps.append(m)
    return in_maps


def kernel(**inputs):
    nc = _get_nc()
    in_maps = _shard_inputs(inputs)
    trace = os.environ.get("KERNEL_PROFILE", "") == "1"
    res = run_bass_kernel_spmd(
        nc, in_maps, core_ids=list(range(N_CORES)), trace=trace
    )
    KSTATS["exec_time_ns"] = res.exec_time_ns
    return _assemble({c: res.results[c] for c in range(N_CORES)})


RS_PAIRS_HOST = [(7, 0), (6, 1), (5, 2), (4, 3)]


def _assemble(results):
    # pair-chunked reduce-scatter: chunk p holds [ti_a(128 rows) | ti_b(128)];
    # RS gives rank r the contiguous 64-row strip r of that 256-row chunk.
    out = np.zeros((B, T, C), np.float32)
    for core in range(N_CORES):
        b, r = core // 4, core % 4
        res = results[core]["out"]
        for p, (ta, tb) in enumerate(RS_PAIRS_HOST):
            ti = ta if r < 2 else tb
            off = (r % 2) * 64
            out[b, ti * 128 + off: ti * 128 + off + 64, :] = \
                res[p * 64:(p + 1) * 64, :]
    return out

